# revision 37
# baseline (speedup 1.0000x reference)
"""Trainium2 Bass kernel for nn_BitFeedForward (BitNet b1.58 FFN).

Math (forward values of the reference):
  x_int  = round(x * 127/max|x|_row)            (ints in [-127,127] -> exact in bf16)
  w_tern = clip(round(w / mean|w|), -1, 1)      (ternary -> exact in fp8e4)
  h_int  = x_int @ w_up_tern^T                  (exact integer math in bf16xfp8 matmul, fp32 PSUM)
  g      = relu(h)^2 ; g_int = round(g * 127/max g_row)
  out    = (g_int @ w_down_tern^T) * mean|w_dn| * maxg_row/127

Sharding: data-parallel over the 16384 tokens (2048 tokens/core) for the
GEMMs; the weight quantization is sharded 8-way over output features
(512 rows of w_up / 256 rows of w_down per core) with an AllReduce for
the global mean|W| partials and two fp8 AllGathers to replicate the
ternary transposed weights to every core.

Block phase is software-pipelined (skewed): Tensor order is
MM1(b), MM2(b-1), MM1(b+1), ... so the activation-quant chain of block b
runs on DVE/ScalarE while the PE does MM2(b-1), keeping the PE dense.
"""
import sys

sys.path.insert(0, "/opt/trn_rl_repo")

import numpy as np
from contextlib import ExitStack

import concourse.bass as bass  # noqa: F401
import concourse.mybir as mybir
import concourse.tile as tile
from concourse import bacc
from concourse.bass_utils import run_bass_kernel_spmd

F32 = mybir.dt.float32
BF16 = mybir.dt.bfloat16
F8 = mybir.dt.float8e4
AX = mybir.AxisListType
OP = mybir.AluOpType
AF = mybir.ActivationFunctionType

N_CORES = 8
B, S, H = 4, 4096, 2048
I = 4096
M_TOT = B * S          # 16384 tokens
M_CORE = M_TOT // N_CORES
P = 128
KUP = H // P           # 16 k-chunks for MM1 (contract over H)
KDN = I // P           # 32 k-chunks for MM2 (contract over I)
C_RND = 12582912.0     # 1.5 * 2**23 : fp32 round-to-nearest-even trick
QB = 127.0
EPS = 1e-5
INV127 = 1.0 / 127.0
WBLK = 1024            # natural weight-load width (f32 elems per partition row)
IUP_SH = I // N_CORES  # 512 w_up rows quantized per core
HDN_SH = H // N_CORES  # 256 w_down rows quantized per core


def build_nc(m_core=M_CORE):
    nblk = m_core // P
    nc = bacc.Bacc("TRN2", target_bir_lowering=False, debug=False)
    x_d = nc.dram_tensor("x", [m_core, H], F32, kind="ExternalInput")
    # weight slices arrive host-transposed in k-major [p, kc, out-col] layout
    wup_d = nc.dram_tensor("w_up_shT", [P, KUP, IUP_SH], F32, kind="ExternalInput")
    wdn_d = nc.dram_tensor("w_dn_shT", [P, KDN, HDN_SH], F32, kind="ExternalInput")
    out_d = nc.dram_tensor("out", [m_core, H], F32, kind="ExternalOutput")
    x_ap, wup_ap, wdn_ap, out_ap = x_d.ap(), wup_d.ap(), wdn_d.ap(), out_d.ap()
    RG = [list(range(N_CORES))]

    with tile.TileContext(nc) as tc, ExitStack() as ctx:
        wres = ctx.enter_context(tc.tile_pool(name="wres", bufs=1))
        wstage = ctx.enter_context(tc.tile_pool(name="wstage", bufs=3))
        hpool = ctx.enter_context(tc.tile_pool(name="hpool", bufs=1))
        xpool = ctx.enter_context(tc.tile_pool(name="xpool", bufs=1))
        xipool = ctx.enter_context(tc.tile_pool(name="xipool", bufs=1))
        xtpool = ctx.enter_context(tc.tile_pool(name="xtpool", bufs=2))
        gtmp = ctx.enter_context(tc.tile_pool(name="gtmp", bufs=3))
        gipool = ctx.enter_context(tc.tile_pool(name="gipool", bufs=2))
        gtpool = ctx.enter_context(tc.tile_pool(name="gtpool", bufs=2))
        opool = ctx.enter_context(tc.tile_pool(name="opool", bufs=2))
        sm = ctx.enter_context(tc.tile_pool(name="sm", bufs=2))
        single = ctx.enter_context(tc.tile_pool(name="single", bufs=1))
        psA = ctx.enter_context(tc.tile_pool(name="psA", bufs=4, space="PSUM"))
        psB = ctx.enter_context(tc.tile_pool(name="psB", bufs=4, space="PSUM"))
        dram = ctx.enter_context(tc.tile_pool(name="dram", bufs=1, space="DRAM"))

        # resident quantized transposed weights (fp8 ternary), K-major
        wupT = wres.tile([P, KUP, I], F8, tag="wupT")    # [k-in-chunk, kc, i]
        wdnT = wres.tile([P, KDN, H], F8, tag="wdnT")    # [k-in-chunk, kc, h]
        ones_sb = single.tile([P, P], F32, tag="ones")
        nc.vector.memset(ones_sb, 1.0)
        cbias = single.tile([P, 1], F32, tag="cbias")
        nc.vector.memset(cbias, C_RND)
        pacc = single.tile([P, 16], F32, tag="pacc")

        # ---------- x-side quantization (independent of weights) ----------
        def x_prep(b):
            x_sb = xpool.tile([P, H], F32, tag="x", name=f"x_{b}")
            nc.sync.dma_start(out=x_sb, in_=x_ap[b * P:(b + 1) * P, :])
            mx = sm.tile([P, 1], F32, tag="mx", name=f"mx_{b}")
            nc.vector.tensor_reduce(out=mx, in_=x_sb, axis=AX.X, op=OP.max,
                                    apply_absolute_value=True)
            mxc = sm.tile([P, 1], F32, tag="mxc", name=f"mxc_{b}")
            nc.vector.tensor_scalar(out=mxc, in0=mx, scalar1=EPS, scalar2=None, op0=OP.max)
            rx = sm.tile([P, 1], F32, tag="rx", name=f"rx_{b}")
            nc.vector.reciprocal(out=rx, in_=mxc)
            sclx = sm.tile([P, 1], F32, tag="sclx", name=f"sclx_{b}")
            nc.vector.tensor_scalar(out=sclx, in0=rx, scalar1=QB, scalar2=None, op0=OP.mult)
            nc.vector.tensor_scalar(out=x_sb, in0=x_sb, scalar1=sclx, scalar2=C_RND,
                                    op0=OP.mult, op1=OP.add)
            x_int = xipool.tile([P, H], BF16, tag="xi", name=f"xi_{b}")
            nc.vector.tensor_scalar(out=x_int, in0=x_sb, scalar1=C_RND, scalar2=None,
                                    op0=OP.subtract)
            x_intT = xtpool.tile([P, KUP, P], BF16, tag="xT", name=f"xT_{b}")
            nc.sync.dma_start(out=x_intT, in_=x_int, transpose=True)
            return mxc, x_intT

        # ---------- sharded weight quantization ----------
        # slices are k-major [P, kcs, cols]; process units of kpu k-chunks
        # (kpu*cols == WBLK free elems per unit, 8 units per matrix)
        # spread big DMAs across trigger queues; gpsimd carries the collectives
        DMA_ENGS = [nc.sync, nc.scalar]

        def w_units(w_ap_, kcs, cols):
            kpu = WBLK // cols
            for u in range(kcs // kpu):
                yield u, kpu, w_ap_[:, u * kpu:(u + 1) * kpu, :]

        def weight_pass_a(w_ap_, kcs, cols, col0, label, dma_eng):
            # |w| partial sums of this core's slice into pacc[:, col0:...]
            for idx, kpu, src in w_units(w_ap_, kcs, cols):
                stage = wstage.tile([P, kpu, cols], F32, tag="wstage",
                                    name=f"wsA_{label}_{idx}")
                dma_eng.dma_start(out=stage, in_=src)
                nc.scalar.activation(out=stage, in_=stage, func=AF.Abs,
                                     accum_out=pacc[:, col0 + idx:col0 + idx + 1])

        def stats_ar(col0, label):
            # partial |w| sum of one matrix -> AllReduce (trigger side)
            sums = sm.tile([P, 1], F32, tag=f"wsum_{label}")
            nc.vector.tensor_reduce(out=sums, in_=pacc[:, col0:col0 + 8],
                                    axis=AX.X, op=OP.add)
            ar_in = dram.tile([P, 1], F32, tag=f"ar_in_{label}")
            ar_out = dram.tile([P, 1], F32, tag=f"ar_out_{label}", addr_space="Shared")
            nc.scalar.dma_start(out=ar_in, in_=sums)
            nc.gpsimd.collective_compute(
                "AllReduce", OP.add, replica_groups=RG,
                ins=[ar_in.opt()], outs=[ar_out.opt()])
            return ar_out

        def stats_finish(ar_out, label):
            # broadcast the global sum to all partitions; mean + 1/mean tiles
            gsum = sm.tile([P, 1], F32, tag=f"gsum_{label}")
            nc.scalar.dma_start(out=gsum, in_=ar_out)
            ps = psA.tile([P, 512], F32, tag="psA", name=f"wps_{label}")
            nc.tensor.matmul(ps[:, 0:1], lhsT=ones_sb, rhs=gsum, start=True, stop=True)
            mean_t = sm.tile([P, 1], F32, tag=f"wmean_{label}")
            nc.vector.tensor_scalar(out=mean_t, in0=ps[:, 0:1], scalar1=1.0 / float(I * H),
                                    scalar2=EPS, op0=OP.mult, op1=OP.max)
            rinv_t = sm.tile([P, 1], F32, tag=f"wrinv_{label}")
            nc.vector.reciprocal(out=rinv_t, in_=mean_t)
            return mean_t, rinv_t

        def weight_pass_b(w_ap_, kcs, cols, rinv_ap, ag_in, label):
            # k-major load -> u = w*rinv + C on ScalarE -> v = min(u-C, 1) bf16
            # (DVE) -> ternary fp8 max(v, -1) (DVE) -> per-unit DMA into the
            # AllGather DRAM input (no big SBUF shard buffer)
            for idx, kpu, src in w_units(w_ap_, kcs, cols):
                stage = wstage.tile([P, kpu, cols], F32, tag="wstage",
                                    name=f"wsB_{label}_{idx}")
                nc.sync.dma_start(out=stage, in_=src)
                nc.scalar.activation(out=stage, in_=stage, func=AF.Identity,
                                     bias=cbias, scale=rinv_ap)
                wq = gtmp.tile([P, kpu, cols], BF16, tag="wq", name=f"wq_{label}_{idx}")
                nc.vector.tensor_scalar(out=wq, in0=stage, scalar1=C_RND, scalar2=1.0,
                                        op0=OP.subtract, op1=OP.min)
                w8 = gipool.tile([P, kpu, cols], F8, tag="gi", name=f"w8_{label}_{idx}")
                nc.vector.tensor_scalar(out=w8, in0=wq, scalar1=-1.0, scalar2=None,
                                        op0=OP.max)
                nc.scalar.dma_start(out=ag_in[:, idx * WBLK:(idx + 1) * WBLK], in_=w8)

        def weight_prep(x_prefetch):
            # pass A up on the quiet gpsimd queue; its AllReduce fires while
            # pass A dn (sync-queue loads) still runs
            weight_pass_a(wup_ap, KUP, IUP_SH, 0, "up", nc.gpsimd)
            ar_up_out = stats_ar(0, "up")
            weight_pass_a(wdn_ap, KDN, HDN_SH, 8, "dn", nc.sync)
            ar_dn_out = stats_ar(8, "dn")
            mean_up, rinv_up = stats_finish(ar_up_out, "up")
            # x prefetch emitted here: runs during the AllReduce waits,
            # off the pass A critical path
            x_prefetch()
            ag_up_in = dram.tile([P, KUP * IUP_SH], F8, tag="ag_up_in")
            ag_up_out = dram.tile([N_CORES * P, KUP, IUP_SH], F8, tag="ag_up_out",
                                  addr_space="Shared")
            weight_pass_b(wup_ap, KUP, IUP_SH, rinv_up, ag_up_in, "up")
            nc.gpsimd.collective_compute(
                "AllGather", OP.bypass, replica_groups=RG,
                ins=[ag_up_in.opt()], outs=[ag_up_out.opt()])
            mean_dn, rinv_dn = stats_finish(ar_dn_out, "dn")
            ag_dn_in = dram.tile([P, KDN * HDN_SH], F8, tag="ag_dn_in")
            ag_dn_out = dram.tile([N_CORES * P, KDN, HDN_SH], F8, tag="ag_dn_out",
                                  addr_space="Shared")
            weight_pass_b(wdn_ap, KDN, HDN_SH, rinv_dn, ag_dn_in, "dn")
            nc.gpsimd.collective_compute(
                "AllGather", OP.bypass, replica_groups=RG,
                ins=[ag_dn_in.opt()], outs=[ag_dn_out.opt()])
            # wupT unpack split across sync/scalar (runs as soon as AG_up lands);
            # wdnT unpack on the gpsimd queue, which is idle after the last
            # collective's completion wait and off the block-phase DMA queues.
            for j in range(N_CORES):
                DMA_ENGS[j % 2].dma_start(
                    out=wupT[:, :, j * IUP_SH:(j + 1) * IUP_SH],
                    in_=ag_up_out[j * P:(j + 1) * P, :, :])
            for j in range(N_CORES):
                nc.gpsimd.dma_start(
                    out=wdnT[:, :, j * HDN_SH:(j + 1) * HDN_SH],
                    in_=ag_dn_out[j * P:(j + 1) * P, :, :])
            return mean_up, mean_dn

        # ---------- block phase ----------
        def mm1(b, mxc, x_intT, mean_up, mean_dn):
            """up-proj for block b: 8 n-chunks x 16 k; h -> SBUF; row stats."""
            c1 = sm.tile([P, 1], F32, tag="c1", name=f"c1_{b}")
            nc.vector.tensor_scalar(out=c1, in0=mxc, scalar1=mean_up,
                                    scalar2=INV127, op0=OP.mult, op1=OP.mult)
            h_sb = hpool.tile([P, I], F32, tag="h", name=f"h_{b}")
            for ns in range(I // 512):
                ps = psA.tile([P, 512], F32, tag="psA", name=f"ps1_{b}_{ns}")
                for k in range(KUP):
                    nc.tensor.matmul(ps, lhsT=x_intT[:, k, :],
                                     rhs=wupT[:, k, ns * 512:(ns + 1) * 512],
                                     start=(k == 0), stop=(k == KUP - 1))
                nc.scalar.activation(out=h_sb[:, ns * 512:(ns + 1) * 512], in_=ps, func=AF.Copy)
            hp = sm.tile([P, 1], F32, tag="hp", name=f"hp_{b}")
            nc.vector.tensor_reduce(out=hp, in_=h_sb, axis=AX.X, op=OP.max)
            hr = sm.tile([P, 1], F32, tag="hr", name=f"hr_{b}")
            nc.vector.tensor_scalar(out=hr, in0=hp, scalar1=0.0, scalar2=c1,
                                    op0=OP.max, op1=OP.mult)   # relu(hp)*c1
            gmaxc = sm.tile([P, 1], F32, tag="gmaxc", name=f"gmaxc_{b}")
            nc.vector.tensor_scalar(out=gmaxc, in0=hr, scalar1=hr, scalar2=EPS,
                                    op0=OP.mult, op1=OP.max)   # max(hr^2, EPS)
            rg = sm.tile([P, 1], F32, tag="rg", name=f"rg_{b}")
            nc.vector.reciprocal(out=rg, in_=gmaxc)
            sclg = sm.tile([P, 1], F32, tag="sclg", name=f"sclg_{b}")
            nc.vector.tensor_scalar(out=sclg, in0=rg, scalar1=QB, scalar2=None, op0=OP.mult)
            c1sq = sm.tile([P, 1], F32, tag="c1sq", name=f"c1sq_{b}")
            nc.vector.tensor_scalar(out=c1sq, in0=c1, scalar1=c1, scalar2=None, op0=OP.mult)
            p1sq = sm.tile([P, 1], F32, tag="p1sq", name=f"p1sq_{b}")
            nc.vector.tensor_scalar(out=p1sq, in0=sclg, scalar1=c1sq, scalar2=None, op0=OP.mult)
            corr2 = sm.tile([P, 1], F32, tag="corr2", name=f"corr2_{b}")
            nc.vector.tensor_scalar(out=corr2, in0=gmaxc, scalar1=mean_dn,
                                    scalar2=INV127, op0=OP.mult, op1=OP.mult)
            return h_sb, p1sq, corr2

        def g_quant(b, h_sb, p1sq):
            """relu^2 + act-quant of block b -> transposed bf16 gintT."""
            gintT = gtpool.tile([P, KDN, P], BF16, tag="gT", name=f"gT_{b}")
            for ns in range(I // 512):
                sl = slice(ns * 512, (ns + 1) * 512)
                r2 = gtmp.tile([P, 512], F32, tag="wq", name=f"r2_{b}_{ns}")
                nc.vector.scalar_tensor_tensor(out=r2, in0=h_sb[:, sl], scalar=0.0,
                                               in1=h_sb[:, sl], op0=OP.max, op1=OP.mult)
                nc.scalar.activation(out=r2, in_=r2, func=AF.Identity, bias=cbias, scale=p1sq)
                g_i = gipool.tile([P, 512], BF16, tag="gi", name=f"gi_{b}_{ns}")
                nc.vector.tensor_scalar(out=g_i, in0=r2, scalar1=C_RND, scalar2=None,
                                        op0=OP.subtract)
                nc.sync.dma_start(out=gintT[:, ns * 4:(ns + 1) * 4, :], in_=g_i,
                                  transpose=True)
            return gintT

        def mm2(b, gintT, corr2):
            """down-proj of block b: 4 n-chunks x 32 k; scale + store."""
            for n2 in range(H // 512):
                ps2 = psB.tile([P, 512], F32, tag="psB", name=f"ps2_{b}_{n2}")
                for k in range(KDN):
                    nc.tensor.matmul(ps2, lhsT=gintT[:, k, :],
                                     rhs=wdnT[:, k, n2 * 512:(n2 + 1) * 512],
                                     start=(k == 0), stop=(k == KDN - 1))
                o_sb = opool.tile([P, 512], F32, tag="wqT", name=f"o_{b}_{n2}")
                nc.scalar.activation(out=o_sb, in_=ps2, func=AF.Copy, scale=corr2)
                nc.scalar.dma_start(out=out_ap[b * P:(b + 1) * P, n2 * 512:(n2 + 1) * 512],
                                    in_=o_sb)

        # ---------- emission ----------
        xq = {}

        def x_prefetch():
            for b in range(min(2, nblk)):
                xq[b] = x_prep(b)

        mean_up, mean_dn = weight_prep(x_prefetch)

        # skew-2 software pipeline: Tensor order mm1(0), mm1(1), mm1(2),
        # mm2(0), mm1(3), mm2(1), ... -> wdnT (late AllGather) is first
        # needed ~3 mm1-phases after the block phase starts, and each
        # block's g-quant has two full mm phases of slack.
        SKEW = 2 if nblk > 2 else 1
        pending = []
        for b in range(nblk):
            mxc, x_intT = xq.pop(b)
            h_sb, p1sq, corr2 = mm1(b, mxc, x_intT, mean_up, mean_dn)
            if len(pending) >= SKEW:
                mm2(*pending.pop(0))
            gintT = g_quant(b, h_sb, p1sq)
            pending.append((b, gintT, corr2))
            if b + 2 < nblk:
                xq[b + 2] = x_prep(b + 2)
        for args in pending:
            mm2(*args)

    nc.compile()
    return nc


_NC_CACHE = {}


def _get_nc(m_core=M_CORE):
    if m_core not in _NC_CACHE:
        _NC_CACHE[m_core] = build_nc(m_core)
    return _NC_CACHE[m_core]


def make_in_maps(x, w_up, w_down):
    # weight slices go up host-transposed to k-major [p, kc, out-col]:
    # wupT[p, kc, io] = w_up[i0+io, kc*128+p] (layout only, no arithmetic)
    def kmajor(sl, kcs, cols):
        return np.ascontiguousarray(
            sl.T.reshape(kcs, P, cols).transpose(1, 0, 2))
    return [
        {"x": x[i * M_CORE:(i + 1) * M_CORE],
         "w_up_shT": kmajor(w_up[i * IUP_SH:(i + 1) * IUP_SH], KUP, IUP_SH),
         "w_dn_shT": kmajor(w_down[i * HDN_SH:(i + 1) * HDN_SH], KDN, HDN_SH)}
        for i in range(N_CORES)
    ]


def kernel(hidden_states, w_up, w_down):
    x = np.ascontiguousarray(np.asarray(hidden_states, dtype=np.float32).reshape(M_TOT, H))
    w_up = np.ascontiguousarray(np.asarray(w_up, dtype=np.float32))
    w_down = np.ascontiguousarray(np.asarray(w_down, dtype=np.float32))
    nc = _get_nc()
    in_maps = make_in_maps(x, w_up, w_down)
    for _attempt in range(3):
        res = run_bass_kernel_spmd(nc, in_maps, list(range(N_CORES)))
        out = np.concatenate([res.results[i]["out"] for i in range(N_CORES)], axis=0)
        if np.isfinite(out).all():
            break
    return out.reshape(B, S, H).astype(np.float32)


# revision 40
# speedup vs baseline: 1.1085x; 1.1085x over previous
"""Trainium2 Bass kernel for nn_BitFeedForward (BitNet b1.58 FFN).

Math (forward values of the reference):
  x_int  = round(x * 127/max|x|_row)            (ints in [-127,127] -> exact in bf16)
  w_tern = clip(round(w / mean|w|), -1, 1)      (ternary -> exact in fp8e4)
  h_int  = x_int @ w_up_tern^T                  (exact integer math in bf16xfp8 matmul, fp32 PSUM)
  g      = relu(h)^2 ; g_int = round(g * 127/max g_row)
  out    = (g_int @ w_down_tern^T) * mean|w_dn| * maxg_row/127

Sharding: data-parallel over the 16384 tokens (2048 tokens/core) for the
GEMMs; the weight quantization is sharded 8-way over output features
(512 rows of w_up / 256 rows of w_down per core) with an AllReduce for
the global mean|W| partials and two fp8 AllGathers to replicate the
ternary transposed weights to every core.

Block phase is software-pipelined (skewed): Tensor order is
MM1(b), MM2(b-1), MM1(b+1), ... so the activation-quant chain of block b
runs on DVE/ScalarE while the PE does MM2(b-1), keeping the PE dense.
"""
import sys

sys.path.insert(0, "/opt/trn_rl_repo")

import numpy as np
from contextlib import ExitStack

import concourse.bass as bass  # noqa: F401
import concourse.mybir as mybir
import concourse.tile as tile
from concourse import bacc
from concourse.bass_utils import run_bass_kernel_spmd

F32 = mybir.dt.float32
BF16 = mybir.dt.bfloat16
F8 = mybir.dt.float8e4
AX = mybir.AxisListType
OP = mybir.AluOpType
AF = mybir.ActivationFunctionType

N_CORES = 8
B, S, H = 4, 4096, 2048
I = 4096
M_TOT = B * S          # 16384 tokens
M_CORE = M_TOT // N_CORES
P = 128
KUP = H // P           # 16 k-chunks for MM1 (contract over H)
KDN = I // P           # 32 k-chunks for MM2 (contract over I)
C_RND = 12582912.0     # 1.5 * 2**23 : fp32 round-to-nearest-even trick
QB = 127.0
EPS = 1e-5
INV127 = 1.0 / 127.0
WBLK = 1024            # natural weight-load width (f32 elems per partition row)
IUP_SH = I // N_CORES  # 512 w_up rows quantized per core
HDN_SH = H // N_CORES  # 256 w_down rows quantized per core


def build_nc(m_core=M_CORE):
    nblk = m_core // P
    nc = bacc.Bacc("TRN2", target_bir_lowering=False, debug=False)
    x_d = nc.dram_tensor("x", [m_core, H], F32, kind="ExternalInput")
    # weight slices arrive host-transposed in k-major [p, kc, out-col] layout
    wup_d = nc.dram_tensor("w_up_shT", [P, KUP, IUP_SH], F32, kind="ExternalInput")
    wdn_d = nc.dram_tensor("w_dn_shT", [P, KDN, HDN_SH], F32, kind="ExternalInput")
    out_d = nc.dram_tensor("out", [m_core, H], F32, kind="ExternalOutput")
    x_ap, wup_ap, wdn_ap, out_ap = x_d.ap(), wup_d.ap(), wdn_d.ap(), out_d.ap()
    RG = [list(range(N_CORES))]

    with tile.TileContext(nc) as tc, ExitStack() as ctx:
        wres = ctx.enter_context(tc.tile_pool(name="wres", bufs=1))
        wstage = ctx.enter_context(tc.tile_pool(name="wstage", bufs=3))
        hpool = ctx.enter_context(tc.tile_pool(name="hpool", bufs=1))
        xpool = ctx.enter_context(tc.tile_pool(name="xpool", bufs=1))
        xipool = ctx.enter_context(tc.tile_pool(name="xipool", bufs=1))
        xtpool = ctx.enter_context(tc.tile_pool(name="xtpool", bufs=2))
        gtmp = ctx.enter_context(tc.tile_pool(name="gtmp", bufs=3))
        gipool = ctx.enter_context(tc.tile_pool(name="gipool", bufs=2))
        gtpool = ctx.enter_context(tc.tile_pool(name="gtpool", bufs=2))
        opool = ctx.enter_context(tc.tile_pool(name="opool", bufs=2))
        sm = ctx.enter_context(tc.tile_pool(name="sm", bufs=2))
        single = ctx.enter_context(tc.tile_pool(name="single", bufs=1))
        psA = ctx.enter_context(tc.tile_pool(name="psA", bufs=4, space="PSUM"))
        psB = ctx.enter_context(tc.tile_pool(name="psB", bufs=4, space="PSUM"))
        dram = ctx.enter_context(tc.tile_pool(name="dram", bufs=1, space="DRAM"))

        # resident quantized transposed weights (fp8 ternary), K-major
        wupT = wres.tile([P, KUP, I], F8, tag="wupT")    # [k-in-chunk, kc, i]
        wdnT = wres.tile([P, KDN, H], F8, tag="wdnT")    # [k-in-chunk, kc, h]
        ones_sb = single.tile([P, P], F32, tag="ones")
        nc.vector.memset(ones_sb, 1.0)
        cbias = single.tile([P, 1], F32, tag="cbias")
        nc.vector.memset(cbias, C_RND)
        pacc = single.tile([P, 16], F32, tag="pacc")

        # ---------- x-side quantization (independent of weights) ----------
        def x_prep(b):
            x_sb = xpool.tile([P, H], F32, tag="x", name=f"x_{b}")
            nc.sync.dma_start(out=x_sb, in_=x_ap[b * P:(b + 1) * P, :])
            mx = sm.tile([P, 1], F32, tag="mx", name=f"mx_{b}")
            nc.vector.tensor_reduce(out=mx, in_=x_sb, axis=AX.X, op=OP.max,
                                    apply_absolute_value=True)
            mxc = sm.tile([P, 1], F32, tag="mxc", name=f"mxc_{b}")
            nc.vector.tensor_scalar(out=mxc, in0=mx, scalar1=EPS, scalar2=None, op0=OP.max)
            rx = sm.tile([P, 1], F32, tag="rx", name=f"rx_{b}")
            nc.vector.reciprocal(out=rx, in_=mxc)
            sclx = sm.tile([P, 1], F32, tag="sclx", name=f"sclx_{b}")
            nc.vector.tensor_scalar(out=sclx, in0=rx, scalar1=QB, scalar2=None, op0=OP.mult)
            nc.vector.tensor_scalar(out=x_sb, in0=x_sb, scalar1=sclx, scalar2=C_RND,
                                    op0=OP.mult, op1=OP.add)
            x_int = xipool.tile([P, H], BF16, tag="xi", name=f"xi_{b}")
            nc.vector.tensor_scalar(out=x_int, in0=x_sb, scalar1=C_RND, scalar2=None,
                                    op0=OP.subtract)
            x_intT = xtpool.tile([P, KUP, P], BF16, tag="xT", name=f"xT_{b}")
            nc.sync.dma_start(out=x_intT, in_=x_int, transpose=True)
            return mxc, x_intT

        # ---------- sharded weight quantization ----------
        # slices are k-major [P, kcs, cols]; process units of kpu k-chunks
        # (kpu*cols == WBLK free elems per unit, 8 units per matrix)
        # spread big DMAs across trigger queues; gpsimd carries the collectives
        DMA_ENGS = [nc.sync, nc.scalar]

        def w_units(w_ap_, kcs, cols):
            kpu = WBLK // cols
            for u in range(kcs // kpu):
                yield u, kpu, w_ap_[:, u * kpu:(u + 1) * kpu, :]

        def weight_pass_a(w_ap_, kcs, cols, col0, label, dma_eng):
            # |w| partial sums of this core's slice into pacc[:, col0:...]
            for idx, kpu, src in w_units(w_ap_, kcs, cols):
                stage = wstage.tile([P, kpu, cols], F32, tag="wstage",
                                    name=f"wsA_{label}_{idx}")
                dma_eng.dma_start(out=stage, in_=src)
                nc.scalar.activation(out=stage, in_=stage, func=AF.Abs,
                                     accum_out=pacc[:, col0 + idx:col0 + idx + 1])

        def stats_ar(col0, label):
            # partial |w| sum of one matrix -> AllReduce (trigger side)
            sums = sm.tile([P, 1], F32, tag=f"wsum_{label}")
            nc.vector.tensor_reduce(out=sums, in_=pacc[:, col0:col0 + 8],
                                    axis=AX.X, op=OP.add)
            ar_in = dram.tile([P, 1], F32, tag=f"ar_in_{label}")
            ar_out = dram.tile([P, 1], F32, tag=f"ar_out_{label}", addr_space="Shared")
            nc.scalar.dma_start(out=ar_in, in_=sums)
            nc.gpsimd.collective_compute(
                "AllReduce", OP.add, replica_groups=RG,
                ins=[ar_in.opt()], outs=[ar_out.opt()])
            return ar_out

        def stats_finish(ar_out, label):
            # broadcast the global sum to all partitions; mean + 1/mean tiles
            gsum = sm.tile([P, 1], F32, tag=f"gsum_{label}")
            nc.scalar.dma_start(out=gsum, in_=ar_out)
            ps = psA.tile([P, 512], F32, tag="psA", name=f"wps_{label}")
            nc.tensor.matmul(ps[:, 0:1], lhsT=ones_sb, rhs=gsum, start=True, stop=True)
            mean_t = sm.tile([P, 1], F32, tag=f"wmean_{label}")
            nc.vector.tensor_scalar(out=mean_t, in0=ps[:, 0:1], scalar1=1.0 / float(I * H),
                                    scalar2=EPS, op0=OP.mult, op1=OP.max)
            rinv_t = sm.tile([P, 1], F32, tag=f"wrinv_{label}")
            nc.vector.reciprocal(out=rinv_t, in_=mean_t)
            return mean_t, rinv_t

        def weight_pass_b(w_ap_, kcs, cols, rinv_ap, ag_in, label):
            # k-major load -> u = w*rinv + C on ScalarE -> v = min(u-C, 1) bf16
            # (DVE) -> ternary fp8 max(v, -1) (DVE) -> per-unit DMA into the
            # AllGather DRAM input (no big SBUF shard buffer)
            for idx, kpu, src in w_units(w_ap_, kcs, cols):
                stage = wstage.tile([P, kpu, cols], F32, tag="wstage",
                                    name=f"wsB_{label}_{idx}")
                nc.sync.dma_start(out=stage, in_=src)
                nc.scalar.activation(out=stage, in_=stage, func=AF.Identity,
                                     bias=cbias, scale=rinv_ap)
                wq = gtmp.tile([P, kpu, cols], BF16, tag="wq", name=f"wq_{label}_{idx}")
                nc.vector.tensor_scalar(out=wq, in0=stage, scalar1=C_RND, scalar2=1.0,
                                        op0=OP.subtract, op1=OP.min)
                w8 = gipool.tile([P, kpu, cols], F8, tag="gi", name=f"w8_{label}_{idx}")
                nc.vector.tensor_scalar(out=w8, in0=wq, scalar1=-1.0, scalar2=None,
                                        op0=OP.max)
                nc.scalar.dma_start(out=ag_in[:, idx * WBLK:(idx + 1) * WBLK], in_=w8)

        def weight_prep(x_prefetch):
            # dummy AllReduce at t=0: absorbs cross-core launch skew and ncfw
            # warmup so the real stats collectives see aligned peers
            warm_in = dram.tile([1, 8], F32, tag="warm_in")
            warm_out = dram.tile([1, 8], F32, tag="warm_out", addr_space="Shared")
            warm_sb = sm.tile([1, 8], F32, tag="warm")
            nc.vector.memset(warm_sb, 0.0)
            nc.scalar.dma_start(out=warm_in, in_=warm_sb)
            nc.gpsimd.collective_compute(
                "AllReduce", OP.add, replica_groups=RG,
                ins=[warm_in.opt()], outs=[warm_out.opt()])
            # pass A up loads on the scalar queue (gpsimd is blocked on the
            # warmup collective); its AllReduce fires while pass A dn
            # (sync-queue loads) still runs
            weight_pass_a(wup_ap, KUP, IUP_SH, 0, "up", nc.scalar)
            ar_up_out = stats_ar(0, "up")
            weight_pass_a(wdn_ap, KDN, HDN_SH, 8, "dn", nc.sync)
            ar_dn_out = stats_ar(8, "dn")
            mean_up, rinv_up = stats_finish(ar_up_out, "up")
            # x prefetch emitted here: runs during the AllReduce waits,
            # off the pass A critical path
            x_prefetch()
            ag_up_in = dram.tile([P, KUP * IUP_SH], F8, tag="ag_up_in")
            ag_up_out = dram.tile([N_CORES * P, KUP, IUP_SH], F8, tag="ag_up_out",
                                  addr_space="Shared")
            weight_pass_b(wup_ap, KUP, IUP_SH, rinv_up, ag_up_in, "up")
            nc.gpsimd.collective_compute(
                "AllGather", OP.bypass, replica_groups=RG,
                ins=[ag_up_in.opt()], outs=[ag_up_out.opt()])
            mean_dn, rinv_dn = stats_finish(ar_dn_out, "dn")
            ag_dn_in = dram.tile([P, KDN * HDN_SH], F8, tag="ag_dn_in")
            ag_dn_out = dram.tile([N_CORES * P, KDN, HDN_SH], F8, tag="ag_dn_out",
                                  addr_space="Shared")
            weight_pass_b(wdn_ap, KDN, HDN_SH, rinv_dn, ag_dn_in, "dn")
            nc.gpsimd.collective_compute(
                "AllGather", OP.bypass, replica_groups=RG,
                ins=[ag_dn_in.opt()], outs=[ag_dn_out.opt()])
            # wupT unpack split across sync/scalar (runs as soon as AG_up lands);
            # wdnT unpack on the gpsimd queue, which is idle after the last
            # collective's completion wait and off the block-phase DMA queues.
            for j in range(N_CORES):
                DMA_ENGS[j % 2].dma_start(
                    out=wupT[:, :, j * IUP_SH:(j + 1) * IUP_SH],
                    in_=ag_up_out[j * P:(j + 1) * P, :, :])
            for j in range(N_CORES):
                nc.gpsimd.dma_start(
                    out=wdnT[:, :, j * HDN_SH:(j + 1) * HDN_SH],
                    in_=ag_dn_out[j * P:(j + 1) * P, :, :])
            return mean_up, mean_dn

        # ---------- block phase ----------
        def mm1(b, mxc, x_intT, mean_up, mean_dn):
            """up-proj for block b: 8 n-chunks x 16 k; h -> SBUF; row stats."""
            c1 = sm.tile([P, 1], F32, tag="c1", name=f"c1_{b}")
            nc.vector.tensor_scalar(out=c1, in0=mxc, scalar1=mean_up,
                                    scalar2=INV127, op0=OP.mult, op1=OP.mult)
            h_sb = hpool.tile([P, I], F32, tag="h", name=f"h_{b}")
            for ns in range(I // 512):
                ps = psA.tile([P, 512], F32, tag="psA", name=f"ps1_{b}_{ns}")
                for k in range(KUP):
                    nc.tensor.matmul(ps, lhsT=x_intT[:, k, :],
                                     rhs=wupT[:, k, ns * 512:(ns + 1) * 512],
                                     start=(k == 0), stop=(k == KUP - 1))
                nc.scalar.activation(out=h_sb[:, ns * 512:(ns + 1) * 512], in_=ps, func=AF.Copy)
            hp = sm.tile([P, 1], F32, tag="hp", name=f"hp_{b}")
            nc.vector.tensor_reduce(out=hp, in_=h_sb, axis=AX.X, op=OP.max)
            hr = sm.tile([P, 1], F32, tag="hr", name=f"hr_{b}")
            nc.vector.tensor_scalar(out=hr, in0=hp, scalar1=0.0, scalar2=c1,
                                    op0=OP.max, op1=OP.mult)   # relu(hp)*c1
            gmaxc = sm.tile([P, 1], F32, tag="gmaxc", name=f"gmaxc_{b}")
            nc.vector.tensor_scalar(out=gmaxc, in0=hr, scalar1=hr, scalar2=EPS,
                                    op0=OP.mult, op1=OP.max)   # max(hr^2, EPS)
            rg = sm.tile([P, 1], F32, tag="rg", name=f"rg_{b}")
            nc.vector.reciprocal(out=rg, in_=gmaxc)
            sclg = sm.tile([P, 1], F32, tag="sclg", name=f"sclg_{b}")
            nc.vector.tensor_scalar(out=sclg, in0=rg, scalar1=QB, scalar2=None, op0=OP.mult)
            c1sq = sm.tile([P, 1], F32, tag="c1sq", name=f"c1sq_{b}")
            nc.vector.tensor_scalar(out=c1sq, in0=c1, scalar1=c1, scalar2=None, op0=OP.mult)
            p1sq = sm.tile([P, 1], F32, tag="p1sq", name=f"p1sq_{b}")
            nc.vector.tensor_scalar(out=p1sq, in0=sclg, scalar1=c1sq, scalar2=None, op0=OP.mult)
            corr2 = sm.tile([P, 1], F32, tag="corr2", name=f"corr2_{b}")
            nc.vector.tensor_scalar(out=corr2, in0=gmaxc, scalar1=mean_dn,
                                    scalar2=INV127, op0=OP.mult, op1=OP.mult)
            return h_sb, p1sq, corr2

        def g_quant(b, h_sb, p1sq):
            """relu^2 + act-quant of block b -> transposed bf16 gintT.

            relu(h)^2 is computed in place over h_sb (h is dead once the row
            max is taken); all 8 stt ops are emitted back-to-back so the
            DVE -> ScalarE -> DVE -> DMA chain pipelines across chunks
            instead of ping-ponging serially.
            """
            gintT = gtpool.tile([P, KDN, P], BF16, tag="gT", name=f"gT_{b}")
            for ns in range(I // 512):
                sl = slice(ns * 512, (ns + 1) * 512)
                nc.vector.scalar_tensor_tensor(out=h_sb[:, sl], in0=h_sb[:, sl],
                                               scalar=0.0, in1=h_sb[:, sl],
                                               op0=OP.max, op1=OP.mult)
            for ns in range(I // 512):
                sl = slice(ns * 512, (ns + 1) * 512)
                nc.scalar.activation(out=h_sb[:, sl], in_=h_sb[:, sl], func=AF.Identity,
                                     bias=cbias, scale=p1sq)
                g_i = gipool.tile([P, 512], BF16, tag="gi", name=f"gi_{b}_{ns}")
                nc.vector.tensor_scalar(out=g_i, in0=h_sb[:, sl], scalar1=C_RND,
                                        scalar2=None, op0=OP.subtract)
                nc.sync.dma_start(out=gintT[:, ns * 4:(ns + 1) * 4, :], in_=g_i,
                                  transpose=True)
            return gintT

        def mm2(b, gintT, corr2):
            """down-proj of block b: 4 n-chunks x 32 k; scale + store."""
            for n2 in range(H // 512):
                ps2 = psB.tile([P, 512], F32, tag="psB", name=f"ps2_{b}_{n2}")
                for k in range(KDN):
                    nc.tensor.matmul(ps2, lhsT=gintT[:, k, :],
                                     rhs=wdnT[:, k, n2 * 512:(n2 + 1) * 512],
                                     start=(k == 0), stop=(k == KDN - 1))
                o_sb = opool.tile([P, 512], F32, tag="wqT", name=f"o_{b}_{n2}")
                nc.scalar.activation(out=o_sb, in_=ps2, func=AF.Copy, scale=corr2)
                nc.scalar.dma_start(out=out_ap[b * P:(b + 1) * P, n2 * 512:(n2 + 1) * 512],
                                    in_=o_sb)

        # ---------- emission ----------
        xq = {}

        def x_prefetch():
            for b in range(min(2, nblk)):
                xq[b] = x_prep(b)

        mean_up, mean_dn = weight_prep(x_prefetch)

        # skew-2 software pipeline: Tensor order mm1(0), mm1(1), mm1(2),
        # mm2(0), mm1(3), mm2(1), ... -> wdnT (late AllGather) is first
        # needed ~3 mm1-phases after the block phase starts, and each
        # block's g-quant has two full mm phases of slack.
        SKEW = 2 if nblk > 2 else 1
        pending = []
        for b in range(nblk):
            mxc, x_intT = xq.pop(b)
            h_sb, p1sq, corr2 = mm1(b, mxc, x_intT, mean_up, mean_dn)
            if len(pending) >= SKEW:
                mm2(*pending.pop(0))
            gintT = g_quant(b, h_sb, p1sq)
            pending.append((b, gintT, corr2))
            if b + 2 < nblk:
                xq[b + 2] = x_prep(b + 2)
        for args in pending:
            mm2(*args)

    nc.compile()
    return nc


_NC_CACHE = {}


def _get_nc(m_core=M_CORE):
    if m_core not in _NC_CACHE:
        _NC_CACHE[m_core] = build_nc(m_core)
    return _NC_CACHE[m_core]


def make_in_maps(x, w_up, w_down):
    # weight slices go up host-transposed to k-major [p, kc, out-col]:
    # wupT[p, kc, io] = w_up[i0+io, kc*128+p] (layout only, no arithmetic)
    def kmajor(sl, kcs, cols):
        return np.ascontiguousarray(
            sl.T.reshape(kcs, P, cols).transpose(1, 0, 2))
    return [
        {"x": x[i * M_CORE:(i + 1) * M_CORE],
         "w_up_shT": kmajor(w_up[i * IUP_SH:(i + 1) * IUP_SH], KUP, IUP_SH),
         "w_dn_shT": kmajor(w_down[i * HDN_SH:(i + 1) * HDN_SH], KDN, HDN_SH)}
        for i in range(N_CORES)
    ]


def kernel(hidden_states, w_up, w_down):
    x = np.ascontiguousarray(np.asarray(hidden_states, dtype=np.float32).reshape(M_TOT, H))
    w_up = np.ascontiguousarray(np.asarray(w_up, dtype=np.float32))
    w_down = np.ascontiguousarray(np.asarray(w_down, dtype=np.float32))
    nc = _get_nc()
    in_maps = make_in_maps(x, w_up, w_down)
    for _attempt in range(3):
        res = run_bass_kernel_spmd(nc, in_maps, list(range(N_CORES)))
        out = np.concatenate([res.results[i]["out"] for i in range(N_CORES)], axis=0)
        if np.isfinite(out).all():
            break
    return out.reshape(B, S, H).astype(np.float32)
